# revision 29
# baseline (speedup 1.0000x reference)
"""ANFIS forward kernel for 8 TRN2 NeuronCores (Bass/Tile, SPMD data-parallel).

Math: the Gaussian-membership product over dims is rewritten as a matmul:
    strengths[n,r] = prod_d exp(-(x_nd-a_rd)^2 / (2 b_rd^2))
                   = exp( sum_d 2*a*w*x  - sum_d w*a^2  - sum_d w*x^2 )   with w = 1/(2 b^2)
so logits = [X | 1 | X^2] @ PL  with PL = [2aw ; -const ; -w]  (K = 65 contraction),
and rule_out = [X | 1] @ [Cw ; cb].  Both ride in ONE matmul per 128-sample chunk
(rhs = [PL | PR], N=256).  Everything downstream is per-partition ops:
exp(+row-sum) on ACT, fused multiply-reduce + normalize on DVE.

Sharding: X is split along N across the 8 cores; a/b/c-derived params replicated.
The host pre-transposes/stacks X so the device never transposes anything.
"""

import os
import sys

import numpy as np

for _p in ("/opt/trn_rl_repo",):
    if _p not in sys.path and os.path.isdir(_p):
        sys.path.insert(0, _p)

N, D, R = 65536, 32, 128
NCORES = 8
NSHARD = N // NCORES          # 8192 samples per core
K = 2 * D + 1                 # 65 = [X | 1 | X^2] stacked contraction dim
CHUNK = 128                   # samples per matmul (M dim)
NCHUNK = NSHARD // CHUNK      # 64
GROUP = 8                     # chunks per output-DMA group
NGROUP = NCHUNK // GROUP      # 8

_CACHE = {}

LAST_EXEC_NS = None
LAST_RESULTS = None


def _build_bass():
    import concourse.bacc as bacc
    import concourse.tile as tile
    from concourse import mybir

    f32 = mybir.dt.float32
    nc = bacc.Bacc()

    xt = nc.declare_dram_parameter("xt", [K, NSHARD], f32, isOutput=False)
    pp = nc.declare_dram_parameter("pp", [K, 2 * R], f32, isOutput=False)
    o_str = nc.declare_dram_parameter("o_str", [NSHARD, R], f32, isOutput=True)
    o_norm = nc.declare_dram_parameter("o_norm", [NSHARD, R], f32, isOutput=True)
    o_pred = nc.declare_dram_parameter("o_pred", [NSHARD], f32, isOutput=True)

    with tile.TileContext(nc) as tc:
        with (
            tc.tile_pool(name="singles", bufs=1) as singles,
            tc.tile_pool(name="groups", bufs=NGROUP) as groups,
            tc.tile_pool(name="small", bufs=NGROUP) as small,
            tc.tile_pool(name="scratch", bufs=8) as scratch,
            tc.tile_pool(name="psum", bufs=8, space="PSUM") as psum_pool,
        ):
            # ---- load inputs -------------------------------------------------
            pp_sb = singles.tile([K, 2 * R], f32)
            nc.sync.dma_start(out=pp_sb[:, :], in_=pp[:, :])
            NPIECE = NGROUP  # one piece per group of chunks
            piece = NSHARD // NPIECE
            xt_pieces = []
            for i in range(NPIECE):
                xp = singles.tile([K, piece], f32, tag=f"xt{i}")
                nc.sync.dma_start(
                    out=xp[:, :],
                    in_=xt[:, i * piece : (i + 1) * piece],
                )
                xt_pieces.append(xp)

            pred_final = singles.tile([128, NCHUNK], f32)
            # With the host's sample permutation, partition p owns DRAM rows
            # [p*64, (p+1)*64), so output DMAs are per-partition contiguous.
            # row n = p*NCHUNK + c:
            o_str_v = o_str[:, :].rearrange("(p c) r -> p c r", p=CHUNK)
            o_norm_v = o_norm[:, :].rearrange("(p c) r -> p c r", p=CHUNK)

            for g in range(NGROUP):
                str_sb = groups.tile([128, GROUP, R], f32, tag="str")
                norm_sb = groups.tile([128, GROUP, R], f32, tag="norm")
                sums_g = small.tile([128, GROUP], f32, tag="sums")
                recip_g = small.tile([128, GROUP], f32, tag="recip")
                pred_raw = small.tile([128, GROUP], f32, tag="praw")

                for j2 in range(GROUP // 2):
                    j = j2 * 2
                    # one PSUM bank holds two chunks: [lg0 | rule0 | lg1 | rule1]
                    psum_t = psum_pool.tile([128, 2, 2, R], f32, tag="ps")
                    for h in range(2):
                        nc.tensor.matmul(
                            psum_t[:, h, :, :],
                            lhsT=xt_pieces[g][:, (j + h) * CHUNK : (j + h + 1) * CHUNK],
                            rhs=pp_sb[:, :],
                            start=True,
                            stop=True,
                        )
                    # strengths = exp(logits), both chunks in one ACT op
                    nc.scalar.activation(
                        out=str_sb[:, j : j + 2, :],
                        in_=psum_t[:, :, 0, :],
                        func=mybir.ActivationFunctionType.Exp,
                    )
                    # t = strengths * rule_out ; pred_raw = row-sum(t)
                    t_scr = scratch.tile([128, 2, R], f32, tag="t")
                    nc.vector.tensor_mul(
                        t_scr, str_sb[:, j : j + 2, :], psum_t[:, :, 1, :]
                    )
                    nc.vector.reduce_sum(
                        out=pred_raw[:, j : j + 2],
                        in_=t_scr,
                        axis=mybir.AxisListType.X,
                    )

                # row-sums of strengths, batched 4 chunks per reduce
                for q in range(GROUP // 4):
                    nc.vector.reduce_sum(
                        out=sums_g[:, q * 4 : (q + 1) * 4],
                        in_=str_sb[:, q * 4 : (q + 1) * 4, :],
                        axis=mybir.AxisListType.X,
                    )
                # recip = 1 / (sums + 1e-8)
                nc.vector.tensor_scalar_add(out=recip_g, in0=sums_g, scalar1=1e-8)
                nc.vector.reciprocal(out=recip_g, in_=recip_g)
                # predictions for the group's 8 chunks
                nc.vector.tensor_mul(
                    pred_final[:, g * GROUP : (g + 1) * GROUP], pred_raw, recip_g
                )
                # normalized = strengths * recip (per-partition scalar)
                for j in range(GROUP):
                    nc.vector.tensor_scalar_mul(
                        out=norm_sb[:, j, :],
                        in0=str_sb[:, j, :],
                        scalar1=recip_g[:, j : j + 1],
                    )

                # ---- group outputs (per-partition contiguous 4KB blocks) ----
                cs = slice(g * GROUP, (g + 1) * GROUP)
                nc.sync.dma_start(out=o_str_v[:, cs, :], in_=str_sb)
                nc.sync.dma_start(out=o_norm_v[:, cs, :], in_=norm_sb)

            nc.sync.dma_start(
                out=o_pred[:].rearrange("(p c) -> p c", p=CHUNK), in_=pred_final[:, :]
            )

    nc.compile()
    return nc


def _host_prep(X, a, b, c):
    X = np.ascontiguousarray(X, dtype=np.float32)
    a = np.asarray(a, dtype=np.float32)
    b = np.asarray(b, dtype=np.float32)
    c = np.asarray(c, dtype=np.float32)

    b_cl = np.maximum(b, 1e-8)
    w = 1.0 / (2.0 * b_cl * b_cl)          # [R, D]
    const = (a * a * w).sum(axis=1)        # [R]

    pp = np.zeros((K, 2 * R), dtype=np.float32)
    pp[0:D, 0:R] = (2.0 * a * w).T
    pp[D, 0:R] = -const
    pp[D + 1 : K, 0:R] = -w.T
    pp[0:D, R : 2 * R] = c[:, :D].T
    pp[D, R : 2 * R] = c[:, D]

    xts = []
    for i in range(NCORES):
        Xs = X[i * NSHARD : (i + 1) * NSHARD]          # [NSHARD, D]
        # Permute so device chunk c / partition p = original sample p*64+c:
        # each partition then owns a contiguous 64-row range of the outputs,
        # making the output DMAs per-partition contiguous.
        Xp = Xs.reshape(CHUNK, NCHUNK, D).transpose(1, 0, 2).reshape(NSHARD, D)
        xt = np.empty((K, NSHARD), dtype=np.float32)
        xt[0:D] = Xp.T
        xt[D] = 1.0
        xt[D + 1 : K] = (Xp * Xp).T
        xts.append(np.ascontiguousarray(xt))
    return xts, pp


def _install_ntff_hook():
    """The agent image's antenv lacks axon_hooks; synthesize it so
    run_bass_kernel_spmd(trace=True) can capture NTFF profiles."""
    import types

    if "antenv.axon_hooks" in sys.modules:
        return
    try:
        sys.path.insert(0, "/root/.axon_site")
        from trn_agent_boot.trn_boot import _ntff_profile_via_ctypes

        hook = _ntff_profile_via_ctypes("/opt/axon/libaxon_pjrt.so")
    except Exception:
        return
    mod = types.ModuleType("antenv.axon_hooks")
    holder = {"h": hook}
    mod.set_axon_ntff_profile_hook = lambda h: holder.__setitem__("h", h)
    mod.get_axon_ntff_profile_hook = lambda: holder.get("h")
    sys.modules["antenv.axon_hooks"] = mod
    import antenv

    antenv.axon_hooks = mod


def kernel(X, a, b, c):
    global LAST_EXEC_NS, LAST_RESULTS
    from concourse.bass_utils import run_bass_kernel_spmd

    if "nc" not in _CACHE:
        _CACHE["nc"] = _build_bass()
    nc = _CACHE["nc"]

    xts, pp = _host_prep(X, a, b, c)
    in_maps = [{"xt": xts[i], "pp": pp} for i in range(NCORES)]

    trace = os.environ.get("KERNEL_TRACE", "0") == "1"
    if trace:
        _install_ntff_hook()
    res = run_bass_kernel_spmd(nc, in_maps, core_ids=list(range(NCORES)), trace=trace)
    LAST_EXEC_NS = res.exec_time_ns
    LAST_RESULTS = res

    preds = np.concatenate([res.results[i]["o_pred"] for i in range(NCORES)], axis=0)
    strs = np.concatenate([res.results[i]["o_str"] for i in range(NCORES)], axis=0)
    norms = np.concatenate([res.results[i]["o_norm"] for i in range(NCORES)], axis=0)
    return (preds, strs, norms)


# revision 30
# speedup vs baseline: 1.0052x; 1.0052x over previous
"""ANFIS forward kernel for 8 TRN2 NeuronCores (Bass/Tile, SPMD data-parallel).

Math: the Gaussian-membership product over dims is rewritten as a matmul:
    strengths[n,r] = prod_d exp(-(x_nd-a_rd)^2 / (2 b_rd^2))
                   = exp( sum_d 2*a*w*x  - sum_d w*a^2  - sum_d w*x^2 )   with w = 1/(2 b^2)
so logits = [X | 1 | X^2] @ PL  with PL = [2aw ; -const ; -w]  (K = 65 contraction),
and rule_out = [X | 1] @ [Cw ; cb].  Both ride in ONE matmul per 128-sample chunk
(rhs = [PL | PR], N=256).  Everything downstream is per-partition ops:
exp(+row-sum) on ACT, fused multiply-reduce + normalize on DVE.

Sharding: X is split along N across the 8 cores; a/b/c-derived params replicated.
The host pre-transposes/stacks X so the device never transposes anything.
"""

import os
import sys

import numpy as np

for _p in ("/opt/trn_rl_repo",):
    if _p not in sys.path and os.path.isdir(_p):
        sys.path.insert(0, _p)

N, D, R = 65536, 32, 128
NCORES = 8
NSHARD = N // NCORES          # 8192 samples per core
K = 2 * D + 1                 # 65 = [X | 1 | X^2] stacked contraction dim
CHUNK = 128                   # samples per matmul (M dim)
NCHUNK = NSHARD // CHUNK      # 64
GROUP = 8                     # chunks per output-DMA group
NGROUP = NCHUNK // GROUP      # 8

_CACHE = {}

LAST_EXEC_NS = None
LAST_RESULTS = None


def _build_bass():
    import concourse.bacc as bacc
    import concourse.tile as tile
    from concourse import mybir

    f32 = mybir.dt.float32
    nc = bacc.Bacc()

    xt = nc.declare_dram_parameter("xt", [K, NSHARD], f32, isOutput=False)
    pp = nc.declare_dram_parameter("pp", [K, 2 * R], f32, isOutput=False)
    o_str = nc.declare_dram_parameter("o_str", [NSHARD, R], f32, isOutput=True)
    o_norm = nc.declare_dram_parameter("o_norm", [NSHARD, R], f32, isOutput=True)
    o_pred = nc.declare_dram_parameter("o_pred", [NSHARD], f32, isOutput=True)

    with tile.TileContext(nc) as tc:
        with (
            tc.tile_pool(name="singles", bufs=1) as singles,
            tc.tile_pool(name="groups", bufs=NGROUP) as groups,
            tc.tile_pool(name="small", bufs=NGROUP) as small,
            tc.tile_pool(name="scratch", bufs=8) as scratch,
            tc.tile_pool(name="psum", bufs=8, space="PSUM") as psum_pool,
        ):
            # ---- load inputs -------------------------------------------------
            pp_sb = singles.tile([K, 2 * R], f32)
            nc.sync.dma_start(out=pp_sb[:, :], in_=pp[:, :])
            NPIECE = NGROUP  # one piece per group of chunks
            piece = NSHARD // NPIECE
            xt_pieces = []
            for i in range(NPIECE):
                xp = singles.tile([K, piece], f32, tag=f"xt{i}")
                nc.sync.dma_start(
                    out=xp[:, :],
                    in_=xt[:, i * piece : (i + 1) * piece],
                )
                xt_pieces.append(xp)

            pred_final = singles.tile([128, NCHUNK], f32)
            # With the host's sample permutation, partition p owns DRAM rows
            # [p*64, (p+1)*64), so output DMAs are per-partition contiguous.
            # row n = p*NCHUNK + c:
            o_str_v = o_str[:, :].rearrange("(p c) r -> p c r", p=CHUNK)
            o_norm_v = o_norm[:, :].rearrange("(p c) r -> p c r", p=CHUNK)

            for g in range(NGROUP):
                str_sb = groups.tile([128, GROUP, R], f32, tag="str")
                norm_sb = groups.tile([128, GROUP, R], f32, tag="norm")
                sums_g = small.tile([128, GROUP], f32, tag="sums")
                recip_g = small.tile([128, GROUP], f32, tag="recip")
                pred_raw = small.tile([128, GROUP], f32, tag="praw")

                for j in range(GROUP):
                    lhsT = xt_pieces[g][:, j * CHUNK : (j + 1) * CHUNK]
                    psum_t = psum_pool.tile([128, 2 * R], f32, tag="ps")
                    nc.tensor.matmul(
                        psum_t, lhsT=lhsT, rhs=pp_sb[:, :], start=True, stop=True
                    )
                    # strengths = exp(logits)
                    nc.scalar.activation(
                        out=str_sb[:, j, :],
                        in_=psum_t[:, 0:R],
                        func=mybir.ActivationFunctionType.Exp,
                    )
                    # t = strengths * rule_out ; pred_raw = row-sum(t)
                    t_scr = scratch.tile([128, R], f32, tag="t")
                    nc.vector.tensor_mul(t_scr, str_sb[:, j, :], psum_t[:, R : 2 * R])
                    nc.vector.reduce_sum(
                        out=pred_raw[:, j : j + 1],
                        in_=t_scr,
                        axis=mybir.AxisListType.X,
                    )

                # row-sums of strengths, batched 4 chunks per reduce
                for q in range(GROUP // 4):
                    nc.vector.reduce_sum(
                        out=sums_g[:, q * 4 : (q + 1) * 4],
                        in_=str_sb[:, q * 4 : (q + 1) * 4, :],
                        axis=mybir.AxisListType.X,
                    )
                # recip = 1 / (sums + 1e-8)
                nc.vector.tensor_scalar_add(out=recip_g, in0=sums_g, scalar1=1e-8)
                nc.vector.reciprocal(out=recip_g, in_=recip_g)
                # predictions for the group's 8 chunks
                nc.vector.tensor_mul(
                    pred_final[:, g * GROUP : (g + 1) * GROUP], pred_raw, recip_g
                )
                # normalized = strengths * recip (per-partition scalar)
                for j in range(GROUP):
                    nc.vector.tensor_scalar_mul(
                        out=norm_sb[:, j, :],
                        in0=str_sb[:, j, :],
                        scalar1=recip_g[:, j : j + 1],
                    )

                # ---- group outputs (per-partition contiguous 4KB blocks) ----
                cs = slice(g * GROUP, (g + 1) * GROUP)
                nc.sync.dma_start(out=o_str_v[:, cs, :], in_=str_sb)
                nc.sync.dma_start(out=o_norm_v[:, cs, :], in_=norm_sb)

            nc.sync.dma_start(
                out=o_pred[:].rearrange("(p c) -> p c", p=CHUNK), in_=pred_final[:, :]
            )

    nc.compile()
    return nc


def _host_prep(X, a, b, c):
    X = np.ascontiguousarray(X, dtype=np.float32)
    a = np.asarray(a, dtype=np.float32)
    b = np.asarray(b, dtype=np.float32)
    c = np.asarray(c, dtype=np.float32)

    b_cl = np.maximum(b, 1e-8)
    w = 1.0 / (2.0 * b_cl * b_cl)          # [R, D]
    const = (a * a * w).sum(axis=1)        # [R]

    pp = np.zeros((K, 2 * R), dtype=np.float32)
    pp[0:D, 0:R] = (2.0 * a * w).T
    pp[D, 0:R] = -const
    pp[D + 1 : K, 0:R] = -w.T
    pp[0:D, R : 2 * R] = c[:, :D].T
    pp[D, R : 2 * R] = c[:, D]

    xts = []
    for i in range(NCORES):
        Xs = X[i * NSHARD : (i + 1) * NSHARD]          # [NSHARD, D]
        # Permute so device chunk c / partition p = original sample p*64+c:
        # each partition then owns a contiguous 64-row range of the outputs,
        # making the output DMAs per-partition contiguous.
        Xp = Xs.reshape(CHUNK, NCHUNK, D).transpose(1, 0, 2).reshape(NSHARD, D)
        xt = np.empty((K, NSHARD), dtype=np.float32)
        xt[0:D] = Xp.T
        xt[D] = 1.0
        xt[D + 1 : K] = (Xp * Xp).T
        xts.append(np.ascontiguousarray(xt))
    return xts, pp


def _install_ntff_hook():
    """The agent image's antenv lacks axon_hooks; synthesize it so
    run_bass_kernel_spmd(trace=True) can capture NTFF profiles."""
    import types

    if "antenv.axon_hooks" in sys.modules:
        return
    try:
        sys.path.insert(0, "/root/.axon_site")
        from trn_agent_boot.trn_boot import _ntff_profile_via_ctypes

        hook = _ntff_profile_via_ctypes("/opt/axon/libaxon_pjrt.so")
    except Exception:
        return
    mod = types.ModuleType("antenv.axon_hooks")
    holder = {"h": hook}
    mod.set_axon_ntff_profile_hook = lambda h: holder.__setitem__("h", h)
    mod.get_axon_ntff_profile_hook = lambda: holder.get("h")
    sys.modules["antenv.axon_hooks"] = mod
    import antenv

    antenv.axon_hooks = mod


def kernel(X, a, b, c):
    global LAST_EXEC_NS, LAST_RESULTS
    from concourse.bass_utils import run_bass_kernel_spmd

    if "nc" not in _CACHE:
        _CACHE["nc"] = _build_bass()
    nc = _CACHE["nc"]

    xts, pp = _host_prep(X, a, b, c)
    in_maps = [{"xt": xts[i], "pp": pp} for i in range(NCORES)]

    trace = os.environ.get("KERNEL_TRACE", "0") == "1"
    if trace:
        _install_ntff_hook()
    res = run_bass_kernel_spmd(nc, in_maps, core_ids=list(range(NCORES)), trace=trace)
    LAST_EXEC_NS = res.exec_time_ns
    LAST_RESULTS = res

    preds = np.concatenate([res.results[i]["o_pred"] for i in range(NCORES)], axis=0)
    strs = np.concatenate([res.results[i]["o_str"] for i in range(NCORES)], axis=0)
    norms = np.concatenate([res.results[i]["o_norm"] for i in range(NCORES)], axis=0)
    return (preds, strs, norms)


# revision 38
# speedup vs baseline: 1.1032x; 1.0975x over previous
"""ANFIS forward kernel for 8 TRN2 NeuronCores (Bass/Tile, SPMD data-parallel).

Math: the Gaussian-membership product over dims is rewritten as a matmul:
    strengths[n,r] = prod_d exp(-(x_nd-a_rd)^2 / (2 b_rd^2))
                   = exp( sum_d 2*a*w*x  - sum_d w*a^2  - sum_d w*x^2 )   with w = 1/(2 b^2)
so logits = [X | 1 | X^2] @ PL  with PL = [2aw ; -const ; -w]  (K = 65 contraction),
and rule_out = [X | 1] @ [Cw ; cb].  Both ride in ONE matmul per 128-sample chunk
(rhs = [PL | PR], N=256).  Everything downstream is per-partition ops:
exp(+row-sum) on ACT, fused multiply-reduce + normalize on DVE.

Sharding: X is split along N across the 8 cores; a/b/c-derived params replicated.
The host pre-transposes/stacks X so the device never transposes anything.
"""

import os
import sys

import numpy as np

for _p in ("/opt/trn_rl_repo",):
    if _p not in sys.path and os.path.isdir(_p):
        sys.path.insert(0, _p)

N, D, R = 65536, 32, 128
NCORES = 8
NSHARD = N // NCORES          # 8192 samples per core
K = 2 * D + 1                 # 65 = [X | 1 | X^2] stacked contraction dim
CHUNK = 128                   # samples per matmul (M dim)
NCHUNK = NSHARD // CHUNK      # 64
GROUP = 8                     # chunks per output-DMA group
NGROUP = NCHUNK // GROUP      # 8

_CACHE = {}

LAST_EXEC_NS = None
LAST_RESULTS = None


def _build_bass():
    import concourse.bacc as bacc
    import concourse.tile as tile
    from concourse import mybir

    f32 = mybir.dt.float32
    f32r_dt = mybir.dt.float32r
    nc = bacc.Bacc()

    bf16 = mybir.dt.bfloat16
    xt = nc.declare_dram_parameter("xt", [K, NSHARD], f32, isOutput=False)
    pp = nc.declare_dram_parameter("pp", [K, 2 * R], f32, isOutput=False)
    xtb = nc.declare_dram_parameter("xtb", [D + 1, NSHARD], bf16, isOutput=False)
    ppb = nc.declare_dram_parameter("ppb", [D + 1, R], bf16, isOutput=False)
    o_str = nc.declare_dram_parameter("o_str", [NSHARD, R], f32, isOutput=True)
    o_norm = nc.declare_dram_parameter("o_norm", [NSHARD, R], f32, isOutput=True)
    o_pred = nc.declare_dram_parameter("o_pred", [NSHARD], f32, isOutput=True)

    with tile.TileContext(nc) as tc:
        with (
            tc.tile_pool(name="singles", bufs=1) as singles,
            tc.tile_pool(name="groups", bufs=NGROUP) as groups,
            tc.tile_pool(name="small", bufs=NGROUP) as small,
            tc.tile_pool(name="scratch", bufs=8) as scratch,
            tc.tile_pool(name="psum", bufs=8, space="PSUM") as psum_pool,
        ):
            # ---- load inputs -------------------------------------------------
            pp_sb = singles.tile([K, 2 * R], f32)
            nc.sync.dma_start(out=pp_sb[:, :], in_=pp[:, :])
            ppb_sb = singles.tile([D + 1, R], bf16)
            nc.sync.dma_start(out=ppb_sb[:, :], in_=ppb[:, :])
            NPIECE = NGROUP  # one piece per group of chunks
            piece = NSHARD // NPIECE
            xt_pieces = []
            xtb_pieces = []
            for i in range(NPIECE):
                xp = singles.tile([K, piece], f32, tag=f"xt{i}")
                nc.sync.dma_start(
                    out=xp[:, :],
                    in_=xt[:, i * piece : (i + 1) * piece],
                )
                xt_pieces.append(xp)
                xpb = singles.tile([D + 1, piece], bf16, tag=f"xtb{i}")
                nc.sync.dma_start(
                    out=xpb[:, :],
                    in_=xtb[:, i * piece : (i + 1) * piece],
                )
                xtb_pieces.append(xpb)

            pred_final = singles.tile([128, NCHUNK], f32)
            # With the host's sample permutation, partition p owns DRAM rows
            # [p*64, (p+1)*64), so output DMAs are per-partition contiguous.
            # row n = p*NCHUNK + c:
            o_str_v = o_str[:, :].rearrange("(p c) r -> p c r", p=CHUNK)
            o_norm_v = o_norm[:, :].rearrange("(p c) r -> p c r", p=CHUNK)

            for g in range(NGROUP):
                str_sb = groups.tile([128, GROUP, R], f32, tag="str")
                norm_sb = groups.tile([128, GROUP, R], f32, tag="norm")
                sums_g = small.tile([128, GROUP], f32, tag="sums")
                recip_g = small.tile([128, GROUP], f32, tag="recip")
                pred_raw = small.tile([128, GROUP], f32, tag="praw")

                for j in range(GROUP):
                    lhsT = xt_pieces[g][:, j * CHUNK : (j + 1) * CHUNK]
                    psum_t = psum_pool.tile([128, 2 * R], f32, tag="ps")
                    # logits need full fp32 (4 cyc/row); rule_out tolerates
                    # bf16 inputs (1 cyc/row, rel err ~2e-3)
                    nc.tensor.matmul(
                        psum_t[:, 0:R],
                        lhsT=lhsT,
                        rhs=pp_sb[:, 0:R],
                        start=True,
                        stop=True,
                    )
                    nc.tensor.matmul(
                        psum_t[:, R : 2 * R],
                        lhsT=xtb_pieces[g][:, j * CHUNK : (j + 1) * CHUNK],
                        rhs=ppb_sb[:, :],
                        start=True,
                        stop=True,
                    )
                    # strengths = exp(logits)
                    nc.scalar.activation(
                        out=str_sb[:, j, :],
                        in_=psum_t[:, 0:R],
                        func=mybir.ActivationFunctionType.Exp,
                    )
                    # t = strengths * rule_out ; pred_raw = row-sum(t)
                    t_scr = scratch.tile([128, R], f32, tag="t")
                    nc.vector.tensor_mul(t_scr, str_sb[:, j, :], psum_t[:, R : 2 * R])
                    nc.vector.reduce_sum(
                        out=pred_raw[:, j : j + 1],
                        in_=t_scr,
                        axis=mybir.AxisListType.X,
                    )

                # row-sums of strengths, batched 4 chunks per reduce
                for q in range(GROUP // 4):
                    nc.vector.reduce_sum(
                        out=sums_g[:, q * 4 : (q + 1) * 4],
                        in_=str_sb[:, q * 4 : (q + 1) * 4, :],
                        axis=mybir.AxisListType.X,
                    )
                # recip = 1 / (sums + 1e-8)
                nc.vector.tensor_scalar_add(out=recip_g, in0=sums_g, scalar1=1e-8)
                nc.vector.reciprocal(out=recip_g, in_=recip_g)
                # predictions for the group's 8 chunks
                nc.vector.tensor_mul(
                    pred_final[:, g * GROUP : (g + 1) * GROUP], pred_raw, recip_g
                )
                # normalized = strengths * recip (per-partition scalar)
                for j in range(GROUP):
                    nc.vector.tensor_scalar_mul(
                        out=norm_sb[:, j, :],
                        in0=str_sb[:, j, :],
                        scalar1=recip_g[:, j : j + 1],
                    )

                # ---- group outputs (per-partition contiguous 4KB blocks) ----
                cs = slice(g * GROUP, (g + 1) * GROUP)
                nc.sync.dma_start(out=o_str_v[:, cs, :], in_=str_sb)
                nc.sync.dma_start(out=o_norm_v[:, cs, :], in_=norm_sb)

            nc.sync.dma_start(
                out=o_pred[:].rearrange("(p c) -> p c", p=CHUNK), in_=pred_final[:, :]
            )

    nc.compile()
    return nc


def _host_prep(X, a, b, c):
    X = np.ascontiguousarray(X, dtype=np.float32)
    a = np.asarray(a, dtype=np.float32)
    b = np.asarray(b, dtype=np.float32)
    c = np.asarray(c, dtype=np.float32)

    b_cl = np.maximum(b, 1e-8)
    w = 1.0 / (2.0 * b_cl * b_cl)          # [R, D]
    const = (a * a * w).sum(axis=1)        # [R]

    pp = np.zeros((K, 2 * R), dtype=np.float32)
    pp[0:D, 0:R] = (2.0 * a * w).T
    pp[D, 0:R] = -const
    pp[D + 1 : K, 0:R] = -w.T
    pp[0:D, R : 2 * R] = c[:, :D].T
    pp[D, R : 2 * R] = c[:, D]

    import ml_dtypes

    ppb = np.zeros((D + 1, R), dtype=ml_dtypes.bfloat16)
    ppb[0:D] = c[:, :D].T.astype(ml_dtypes.bfloat16)
    ppb[D] = c[:, D].astype(ml_dtypes.bfloat16)

    xts = []
    xtbs = []
    for i in range(NCORES):
        Xs = X[i * NSHARD : (i + 1) * NSHARD]          # [NSHARD, D]
        # Permute so device chunk c / partition p = original sample p*64+c:
        # each partition then owns a contiguous 64-row range of the outputs,
        # making the output DMAs per-partition contiguous.
        Xp = Xs.reshape(CHUNK, NCHUNK, D).transpose(1, 0, 2).reshape(NSHARD, D)
        xt = np.empty((K, NSHARD), dtype=np.float32)
        xt[0:D] = Xp.T
        xt[D] = 1.0
        xt[D + 1 : K] = (Xp * Xp).T
        xts.append(np.ascontiguousarray(xt))
        xtbs.append(np.ascontiguousarray(xt[0 : D + 1].astype(ml_dtypes.bfloat16)))
    return xts, xtbs, pp, ppb


def _install_ntff_hook():
    """The agent image's antenv lacks axon_hooks; synthesize it so
    run_bass_kernel_spmd(trace=True) can capture NTFF profiles."""
    import types

    if "antenv.axon_hooks" in sys.modules:
        return
    try:
        sys.path.insert(0, "/root/.axon_site")
        from trn_agent_boot.trn_boot import _ntff_profile_via_ctypes

        hook = _ntff_profile_via_ctypes("/opt/axon/libaxon_pjrt.so")
    except Exception:
        return
    mod = types.ModuleType("antenv.axon_hooks")
    holder = {"h": hook}
    mod.set_axon_ntff_profile_hook = lambda h: holder.__setitem__("h", h)
    mod.get_axon_ntff_profile_hook = lambda: holder.get("h")
    sys.modules["antenv.axon_hooks"] = mod
    import antenv

    antenv.axon_hooks = mod


def kernel(X, a, b, c):
    global LAST_EXEC_NS, LAST_RESULTS
    from concourse.bass_utils import run_bass_kernel_spmd

    if "nc" not in _CACHE:
        _CACHE["nc"] = _build_bass()
    nc = _CACHE["nc"]

    xts, xtbs, pp, ppb = _host_prep(X, a, b, c)
    in_maps = [
        {"xt": xts[i], "pp": pp, "xtb": xtbs[i], "ppb": ppb} for i in range(NCORES)
    ]

    trace = os.environ.get("KERNEL_TRACE", "0") == "1"
    if trace:
        _install_ntff_hook()
    res = run_bass_kernel_spmd(nc, in_maps, core_ids=list(range(NCORES)), trace=trace)
    LAST_EXEC_NS = res.exec_time_ns
    LAST_RESULTS = res

    preds = np.concatenate([res.results[i]["o_pred"] for i in range(NCORES)], axis=0)
    strs = np.concatenate([res.results[i]["o_str"] for i in range(NCORES)], axis=0)
    norms = np.concatenate([res.results[i]["o_norm"] for i in range(NCORES)], axis=0)
    return (preds, strs, norms)


# revision 40
# speedup vs baseline: 1.3322x; 1.2075x over previous
"""ANFIS forward kernel for 8 TRN2 NeuronCores (Bass/Tile, SPMD data-parallel).

Math: the Gaussian-membership product over dims is rewritten as matmuls:
    strengths[n,r] = prod_d exp(-(x_nd-a_rd)^2 / (2 b_rd^2))
                   = exp( sum_d 2*a*w*x - sum_d w*a^2 - sum_d w*x^2 ),  w = 1/(2 b^2)
so logits = [X | biasA | biasB | X^2] @ PL (K=66 contraction) and
rule_out = same @ [Cw ; cb].  PE fp32 matmuls are weight-load bound (4 cyc/col),
so X and PL are split hi/lo into fp16 (2x11-bit mantissa ~ fp32 precision,
1 cyc/col): logits = Xhi@Phi + Xhi@Plo + Xlo@Phi, with per-row balanced scaling
s_k = sqrt(max|P_k|/max|x_k|) to keep everything in fp16 range (scales cancel
in the products).  The huge -const bias row is split across two rows
(2^15*constA + constB) so each fits fp16.  Verified numerically: rel err
~5.6e-4 vs the fp32 reference (gate 2e-2).

Sharding: X split along N across 8 cores; params replicated.  The host
pre-transposes/stacks X (no on-device transposes) and permutes samples so
partition p owns contiguous output rows [p*64,(p+1)*64) -> all output DMAs are
per-partition contiguous.
"""

import os
import sys

import numpy as np

for _p in ("/opt/trn_rl_repo",):
    if _p not in sys.path and os.path.isdir(_p):
        sys.path.insert(0, _p)

N, D, R = 65536, 32, 128
NCORES = 8
NSHARD = N // NCORES          # 8192 samples per core
K = 2 * D + 2                 # 66 = [X | biasA | biasB | X^2]
CHUNK = 128                   # samples per matmul (M dim)
NCHUNK = NSHARD // CHUNK      # 64
GROUP = 8                     # chunks per output-DMA group
NGROUP = NCHUNK // GROUP      # 8
SUB = 4                       # chunks per PSUM supertile
SCB = float(2.0**15)          # bias row A scale

_CACHE = {}

LAST_EXEC_NS = None
LAST_RESULTS = None


def _build_bass():
    import concourse.bacc as bacc
    import concourse.tile as tile
    from concourse import mybir

    f32 = mybir.dt.float32
    f16 = mybir.dt.float16
    nc = bacc.Bacc()

    xth = nc.declare_dram_parameter("xth", [K, NSHARD], f16, isOutput=False)
    xtl = nc.declare_dram_parameter("xtl", [K, NSHARD], f16, isOutput=False)
    pphi = nc.declare_dram_parameter("pphi", [K, 2 * R], f16, isOutput=False)
    pplo = nc.declare_dram_parameter("pplo", [K, R], f16, isOutput=False)
    o_str = nc.declare_dram_parameter("o_str", [NSHARD, R], f32, isOutput=True)
    o_norm = nc.declare_dram_parameter("o_norm", [NSHARD, R], f32, isOutput=True)
    o_pred = nc.declare_dram_parameter("o_pred", [NSHARD], f32, isOutput=True)

    with tile.TileContext(nc) as tc:
        with (
            tc.tile_pool(name="singles", bufs=1) as singles,
            tc.tile_pool(name="groups", bufs=3) as groups,
            tc.tile_pool(name="small", bufs=4) as small,
            tc.tile_pool(name="scratch", bufs=4) as scratch,
            tc.tile_pool(name="psum", bufs=3, space="PSUM") as psum_pool,
        ):
            # ---- load inputs -------------------------------------------------
            pphi_sb = singles.tile([K, 2 * R], f16)
            nc.sync.dma_start(out=pphi_sb[:, :], in_=pphi[:, :])
            pplo_sb = singles.tile([K, R], f16)
            nc.sync.dma_start(out=pplo_sb[:, :], in_=pplo[:, :])
            NPIECE = NGROUP  # one piece per group of chunks
            piece = NSHARD // NPIECE
            xth_pieces = []
            xtl_pieces = []
            for i in range(NPIECE):
                ph = singles.tile([K, piece], f16, tag=f"xth{i}")
                nc.sync.dma_start(
                    out=ph[:, :], in_=xth[:, i * piece : (i + 1) * piece]
                )
                xth_pieces.append(ph)
                pl = singles.tile([K, piece], f16, tag=f"xtl{i}")
                nc.sync.dma_start(
                    out=pl[:, :], in_=xtl[:, i * piece : (i + 1) * piece]
                )
                xtl_pieces.append(pl)

            pred_final = singles.tile([128, NCHUNK], f32)
            # permuted outputs: DRAM row n = p*NCHUNK + c
            o_str_v = o_str[:, :].rearrange("(p c) r -> p c r", p=CHUNK)
            o_norm_v = o_norm[:, :].rearrange("(p c) r -> p c r", p=CHUNK)

            for g in range(NGROUP):
                str_sb = groups.tile([128, GROUP, R], f32, tag="str")
                norm_sb = groups.tile([128, GROUP, R], f32, tag="norm")
                sums_g = small.tile([128, GROUP], f32, tag="sums")
                recip_g = small.tile([128, GROUP], f32, tag="recip")
                pred_raw = small.tile([128, GROUP], f32, tag="praw")

                for q in range(GROUP // SUB):
                    # supertile: SUB chunks x [logits | rule], half-bank each
                    ps = psum_pool.tile([128, SUB, 2 * R], f32, tag="ps")
                    for j4 in range(SUB):
                        j = q * SUB + j4
                        lh = xth_pieces[g][:, j * CHUNK : (j + 1) * CHUNK]
                        ll = xtl_pieces[g][:, j * CHUNK : (j + 1) * CHUNK]
                        # hi x [Phi | rule]  (N=256)
                        nc.tensor.matmul(
                            ps[:, j4, :], lhsT=lh, rhs=pphi_sb[:, :],
                            start=True, stop=False,
                        )
                        # hi x Plo (accumulate into logits half)
                        nc.tensor.matmul(
                            ps[:, j4, 0:R], lhsT=lh, rhs=pplo_sb[:, :],
                            start=False, stop=False,
                        )
                        # lo x Phi (accumulate into logits half)
                        nc.tensor.matmul(
                            ps[:, j4, 0:R], lhsT=ll,
                            rhs=pphi_sb[:, 0:R],
                            start=False, stop=True, skip_group_check=True,
                        )
                    # strengths = exp(logits) for the whole supertile
                    nc.scalar.activation(
                        out=str_sb[:, q * SUB : (q + 1) * SUB, :],
                        in_=ps[:, :, 0:R],
                        func=mybir.ActivationFunctionType.Exp,
                    )
                    # t = strengths * rule_out ; pred_raw = row-sums
                    t_scr = scratch.tile([128, SUB, R], f32, tag="t")
                    nc.vector.tensor_mul(
                        t_scr,
                        str_sb[:, q * SUB : (q + 1) * SUB, :],
                        ps[:, :, R : 2 * R],
                    )
                    nc.vector.reduce_sum(
                        out=pred_raw[:, q * SUB : (q + 1) * SUB],
                        in_=t_scr,
                        axis=mybir.AxisListType.X,
                    )
                    # row-sums of strengths
                    nc.vector.reduce_sum(
                        out=sums_g[:, q * SUB : (q + 1) * SUB],
                        in_=str_sb[:, q * SUB : (q + 1) * SUB, :],
                        axis=mybir.AxisListType.X,
                    )

                # recip = 1 / (sums + 1e-8)
                nc.vector.tensor_scalar_add(out=recip_g, in0=sums_g, scalar1=1e-8)
                nc.vector.reciprocal(out=recip_g, in_=recip_g)
                # predictions for the group's chunks
                nc.vector.tensor_mul(
                    pred_final[:, g * GROUP : (g + 1) * GROUP], pred_raw, recip_g
                )
                # normalized = strengths * recip, alternating DVE / ACT
                for j in range(GROUP):
                    if j % 2 == 0:
                        nc.vector.tensor_scalar_mul(
                            out=norm_sb[:, j, :],
                            in0=str_sb[:, j, :],
                            scalar1=recip_g[:, j : j + 1],
                        )
                    else:
                        nc.scalar.activation(
                            out=norm_sb[:, j, :],
                            in_=str_sb[:, j, :],
                            func=mybir.ActivationFunctionType.Copy,
                            scale=recip_g[:, j : j + 1],
                        )

                # ---- group outputs (per-partition contiguous 4KB blocks) ----
                cs = slice(g * GROUP, (g + 1) * GROUP)
                nc.sync.dma_start(out=o_str_v[:, cs, :], in_=str_sb)
                nc.sync.dma_start(out=o_norm_v[:, cs, :], in_=norm_sb)

            nc.sync.dma_start(
                out=o_pred[:].rearrange("(p c) -> p c", p=CHUNK), in_=pred_final[:, :]
            )

    nc.compile()
    return nc


def _host_prep(X, a, b, c):
    X = np.ascontiguousarray(X, dtype=np.float32)
    a = np.asarray(a, dtype=np.float32)
    b = np.asarray(b, dtype=np.float32)
    c = np.asarray(c, dtype=np.float32)

    b_cl = np.maximum(b, 1e-8)
    w = 1.0 / (2.0 * b_cl * b_cl)            # [R, D]
    const = (a * a * w).sum(axis=1)          # [R]

    # K=66 rows: 0..31 X, 32 biasA (x-side 2^15), 33 biasB (x-side 1), 34..65 X^2
    constA16 = (-const / SCB).astype(np.float16).astype(np.float32)
    constB = (-const - SCB * constA16).astype(np.float32)

    PL = np.zeros((K, R), np.float32)
    PL[0:D] = (2.0 * a * w).T
    PL[D] = constA16
    PL[D + 1] = constB
    PL[D + 2 : K] = -w.T

    # per-row balanced scale so both fp16 sides stay in range
    xcol_max = np.empty(K, np.float32)
    xabs = np.abs(X).max()
    x2abs = (X * X).max()
    xcol_max[0:D] = np.abs(X).max(axis=0)
    xcol_max[D] = SCB
    xcol_max[D + 1] = 1.0
    xcol_max[D + 2 : K] = (X * X).max(axis=0)
    pmax = np.abs(PL).max(axis=1)
    s = np.sqrt(np.maximum(pmax, 1e-12) / np.maximum(xcol_max, 1e-12)).astype(
        np.float32
    )
    s[D] = 1.0
    s[D + 1] = 1.0

    Phi = (PL / s[:, None]).astype(np.float16)
    Plo = (PL / s[:, None] - Phi.astype(np.float32)).astype(np.float16)

    pphi = np.zeros((K, 2 * R), np.float16)
    pphi[:, 0:R] = Phi
    # rule params (single fp16 pass): rows 0..31 Cw/s, row 33 cb
    pphi[0:D, R : 2 * R] = (c[:, :D].T / s[0:D, None]).astype(np.float16)
    pphi[D + 1, R : 2 * R] = c[:, D].astype(np.float16)
    pplo = np.ascontiguousarray(Plo)

    xths = []
    xtls = []
    for i in range(NCORES):
        Xs = X[i * NSHARD : (i + 1) * NSHARD]
        # permute: device chunk c / partition p = original sample p*64+c
        Xp = Xs.reshape(CHUNK, NCHUNK, D).transpose(1, 0, 2).reshape(NSHARD, D)
        xs = np.empty((K, NSHARD), np.float32)
        xs[0:D] = Xp.T * s[0:D, None]
        xs[D] = SCB
        xs[D + 1] = 1.0
        xs[D + 2 : K] = (Xp * Xp).T * s[D + 2 : K, None]
        xh = xs.astype(np.float16)
        xl = (xs - xh.astype(np.float32)).astype(np.float16)
        xths.append(np.ascontiguousarray(xh))
        xtls.append(np.ascontiguousarray(xl))
    return xths, xtls, pphi, pplo


def _install_ntff_hook():
    """The agent image's antenv lacks axon_hooks; synthesize it so
    run_bass_kernel_spmd(trace=True) can capture NTFF profiles."""
    import types

    if "antenv.axon_hooks" in sys.modules:
        return
    try:
        sys.path.insert(0, "/root/.axon_site")
        from trn_agent_boot.trn_boot import _ntff_profile_via_ctypes

        hook = _ntff_profile_via_ctypes("/opt/axon/libaxon_pjrt.so")
    except Exception:
        return
    mod = types.ModuleType("antenv.axon_hooks")
    holder = {"h": hook}
    mod.set_axon_ntff_profile_hook = lambda h: holder.__setitem__("h", h)
    mod.get_axon_ntff_profile_hook = lambda: holder.get("h")
    sys.modules["antenv.axon_hooks"] = mod
    import antenv

    antenv.axon_hooks = mod


def kernel(X, a, b, c):
    global LAST_EXEC_NS, LAST_RESULTS
    from concourse.bass_utils import run_bass_kernel_spmd

    if "nc" not in _CACHE:
        _CACHE["nc"] = _build_bass()
    nc = _CACHE["nc"]

    xths, xtls, pphi, pplo = _host_prep(X, a, b, c)
    in_maps = [
        {"xth": xths[i], "xtl": xtls[i], "pphi": pphi, "pplo": pplo}
        for i in range(NCORES)
    ]

    trace = os.environ.get("KERNEL_TRACE", "0") == "1"
    if trace:
        _install_ntff_hook()
    res = run_bass_kernel_spmd(nc, in_maps, core_ids=list(range(NCORES)), trace=trace)
    LAST_EXEC_NS = res.exec_time_ns
    LAST_RESULTS = res

    preds = np.concatenate([res.results[i]["o_pred"] for i in range(NCORES)], axis=0)
    strs = np.concatenate([res.results[i]["o_str"] for i in range(NCORES)], axis=0)
    norms = np.concatenate([res.results[i]["o_norm"] for i in range(NCORES)], axis=0)
    return (preds, strs, norms)
